# revision 15
# baseline (speedup 1.0000x reference)
"""Trainium2 Bass kernel for CausalBCNAttention.

Reference math (per batch b):
    Q = x @ W_Q^T ; K = x @ W_K^T
    A = Q @ V ; Kw = (K @ Wb) * mask
    S = cumsum_t(Kw) ; n = cumsum_t(mask)
    out = ((A*S) @ U^T + n*bias) / max(n,1)
    Y = out @ W_O^T

Algebraic refactoring (exact up to fp reassociation):
    A  = x @ (W_Q^T V)            -- WQV precomputed host-side
    Kw = (x*mask) @ (W_K^T Wb)    -- WKW precomputed host-side
    S  = cumsum_t(Kw)
    Y  = ((A/n) (.) S) @ (U^T W_O^T) + bias @ W_O^T
because the /n factor is a per-row scalar and commutes through right-matmuls.
So the device does 3 GEMMs of (4096x1024)@(1024x1024) per batch plus a
prefix-scan, instead of 6 GEMMs.

Device mapping: data-parallel over B across the 8 NeuronCores (core b owns
batch b). All tensors live "transposed" ([feature, t] with t on the free
dimension): the GEMM chain then needs no on-chip transposes and the causal
cumsum is a native DVE prefix scan (tensor_tensor_scan) along the free dim.
Matmuls run in bf16 with fp32 PSUM accumulation; the scan state is fp32.
"""

import numpy as np
import ml_dtypes

B, T, D, R = 8, 4096, 1024, 1024
P = 128
KO = D // P          # 8 contraction subtiles
RO = R // P          # 8 r tiles
EO = D // P          # 8 output-feature tiles
TB = 1024            # t megablock
NB = T // TB         # 4 megablocks
NC_CHUNK = 512       # matmul free-dim chunk (one PSUM bank)
CPB = TB // NC_CHUNK  # chunks per megablock

_bf16 = ml_dtypes.bfloat16

# Perf-experiment knobs (perf tooling overrides these module globals; the
# graded path always uses the defaults).
PSUM_BUFS = (2, 3, 3)
SCAN_AS_COPY = False
KW_FIRST = False
ST_BUFS = 2


def _patch_tile_drain():
    """This container's walrus codegen rejects any instruction carrying more
    than one sync wait ("Too many sync wait commands"). Two fixes:
    (1) TileContext._add_instruction: hoist N>1 scheduler-attached waits onto
        N standalone NoOps emitted just before the instruction on the same
        engine stream (identical semantics: waits happen-before the op).
    (2) TileContext._drain_and_barrier: the tail drain aggregates waits for
        every active proc onto one Drain; split into one Drain per wait."""
    import bass_rust
    import concourse.mybir as mybir
    import concourse.tile as tile
    from concourse.vector_clock import ScopedClock

    if getattr(tile.TileContext, "_waitsplit_patched", False):
        return

    counter = [0]

    def _split_waits_add_instruction(self, inst):
        si = inst.sync_info
        if si is not None and len(si.on_wait) > 1:
            waits = list(si.on_wait)
            for w in waits:
                counter[0] += 1
                nop = mybir.InstNoOp(name=f"waitsplit-{counter[0]}")
                nop.engine = inst.engine
                nop.sync_info = bass_rust.SyncInfo(on_wait=[w], on_update=[])
                self.nc.register_instruction(nop, overwrite=True)
                self.nc.cur_bb.bb.add_instruction(nop)
            si.on_wait = []
            inst.sync_info = si
        self.nc.register_instruction(inst, overwrite=True)
        self.nc.cur_bb.bb.add_instruction(inst)

    def _split_drain_and_barrier(self, tick_clock, wait_clock):
        nc = self.nc
        drain_inst = nc.sync.drain()
        wait_clock.add_sem_waits(
            drain_inst.ins, ScopedClock({None: tick_clock.global_clock})
        )
        si = drain_inst.ins.sync_info
        waits = list(si.on_wait) if si is not None else []
        if len(waits) > 1:
            si.on_wait = waits[:1]
            drain_inst.ins.sync_info = si
            for w in waits[1:]:
                extra = nc.sync.drain()
                extra.ins.sync_info = bass_rust.SyncInfo(on_wait=[w], on_update=[])

        nc.all_engine_barrier()
        assert self.sems is not None
        popped = nc._tile_sem_poison_stack.pop()
        assert popped is self._sem_poison
        nc.clear_and_free_semaphores(list(self.sems.allocated().values()))
        nc.all_engine_barrier()

    tile.TileContext._add_instruction = _split_waits_add_instruction
    tile.TileContext._drain_and_barrier = _split_drain_and_barrier
    tile.TileContext._waitsplit_patched = True


def _build_program(dual_x: bool):
    """Trace the Bass/Tile program (identical on every core; SPMD)."""
    import concourse.bass as bass
    import concourse.mybir as mybir
    import concourse.tile as tile

    _patch_tile_drain()
    f32 = mybir.dt.float32
    bf16 = mybir.dt.bfloat16
    MUL = mybir.AluOpType.mult
    ADD = mybir.AluOpType.add
    BYP = mybir.AluOpType.bypass

    nc = bass.Bass()
    xt = nc.dram_tensor("xt", [D, T], bf16, kind="ExternalInput")
    xtk = (
        nc.dram_tensor("xtk", [D, T], bf16, kind="ExternalInput") if dual_x else xt
    )
    wa = nc.dram_tensor("wa", [D, R], bf16, kind="ExternalInput")
    wk = nc.dram_tensor("wk", [D, R], bf16, kind="ExternalInput")
    wy = nc.dram_tensor("wy", [R, D], bf16, kind="ExternalInput")
    invn = nc.dram_tensor("invn", [P, T], f32, kind="ExternalInput")
    ybias = nc.dram_tensor("ybias", [P, EO], f32, kind="ExternalInput")
    yt = nc.dram_tensor("yt", [D, T], f32, kind="ExternalOutput")

    xt_r = xt.rearrange("(ko ki) t -> ki ko t", ki=P)
    xtk_r = xtk.rearrange("(ko ki) t -> ki ko t", ki=P)
    wa_r = wa.rearrange("(ko ki) r -> ki ko r", ki=P)
    wk_r = wk.rearrange("(ko ki) r -> ki ko r", ki=P)
    wy_r = wy.rearrange("(ro ri) e -> ri ro e", ri=P)
    yt_r = yt.rearrange("(eo ei) t -> ei eo t", ei=P)

    with tile.TileContext(nc) as tc:
        with (
            tc.tile_pool(name="weights", bufs=1) as wpool,
            tc.tile_pool(name="consts", bufs=1) as cpool,
            tc.tile_pool(name="xin", bufs=2) as xpool,
            tc.tile_pool(name="xkin", bufs=2 if dual_x else 1) as xkpool,
            # dual_x adds 32KB/partition of xtk tiles; shrink the A' pool to
            # stay under the 192KB/partition SBUF budget on that (cold) path.
            tc.tile_pool(name="aprime", bufs=1 if dual_x else 2) as apool,
            tc.tile_pool(name="zbuf", bufs=2) as zpool,
            tc.tile_pool(name="stbuf", bufs=ST_BUFS) as stpool,
            tc.tile_pool(name="carry", bufs=1) as crpool,
            tc.tile_pool(name="yout", bufs=2 if dual_x else 4) as ypool,
            tc.tile_pool(name="pa", bufs=PSUM_BUFS[0], space="PSUM") as pa_pool,
            tc.tile_pool(name="pk", bufs=PSUM_BUFS[1], space="PSUM") as pk_pool,
            tc.tile_pool(name="py", bufs=PSUM_BUFS[2], space="PSUM") as py_pool,
        ):
            # Preamble loads are sliced per contraction subtile so the first
            # matmuls (which need only wa[:, 0] + xt0[:, 0]) can start after
            # <1 MB of DMA instead of after every weight has landed.
            wa_sb = wpool.tile([P, KO, R], bf16, tag="wa")
            wk_sb = wpool.tile([P, KO, R], bf16, tag="wk")
            wy_sb = wpool.tile([P, RO, D], bf16, tag="wy")
            xt0 = xpool.tile([P, KO, TB], bf16, tag="xt", name="xt0")
            invn_sb = cpool.tile([P, T], f32, tag="invn")
            for d in range(KO):
                nc.sync.dma_start(wa_sb[:, d], wa_r[:, d])
                nc.sync.dma_start(xt0[:, d], xt_r[:, d, 0:TB])
                nc.sync.dma_start(wk_sb[:, d], wk_r[:, d])
                nc.sync.dma_start(
                    invn_sb[:, d * (T // KO) : (d + 1) * (T // KO)],
                    invn[:, d * (T // KO) : (d + 1) * (T // KO)],
                )
            nc.sync.dma_start(wy_sb[:], wy_r[:])
            ybias_sb = cpool.tile([P, EO], f32, tag="ybias")
            nc.sync.dma_start(ybias_sb[:], ybias[:])
            carry_sb = crpool.tile([P, RO], f32, tag="carry")

            for mb in range(NB):
                t0 = mb * TB

                if mb == 0:
                    xt_mb = xt0
                else:
                    xt_mb = xpool.tile([P, KO, TB], bf16, tag="xt")
                    nc.sync.dma_start(xt_mb[:], xt_r[:, :, t0 : t0 + TB])
                if dual_x:
                    xtk_mb = xkpool.tile([P, KO, TB], bf16, tag="xtk")
                    nc.sync.dma_start(xtk_mb[:], xtk_r[:, :, t0 : t0 + TB])
                else:
                    xtk_mb = xt_mb

                # ---- A' stage: A'[r,t] = (x @ WQV)^T[r,t] * invn[t] ----
                at_mb = apool.tile([P, RO, TB], bf16, tag="at")

                def a_stage(ro):
                    ps = [
                        pa_pool.tile([P, NC_CHUNK], f32, tag="pa", name=f"pa{c}") for c in range(CPB)
                    ]
                    for d in range(KO):
                        for c in range(CPB):
                            nc.tensor.matmul(
                                ps[c][:],
                                wa_sb[:, d, ro * P : (ro + 1) * P],
                                xt_mb[:, d, c * NC_CHUNK : (c + 1) * NC_CHUNK],
                                start=(d == 0),
                                stop=(d == KO - 1),
                            )
                    for c in range(CPB):
                        cs = slice(c * NC_CHUNK, (c + 1) * NC_CHUNK)
                        gs = slice(t0 + c * NC_CHUNK, t0 + (c + 1) * NC_CHUNK)
                        nc.vector.tensor_tensor(
                            at_mb[:, ro, cs], ps[c][:], invn_sb[:, gs], MUL
                        )

                # ---- Kw stage + causal prefix scan + Z ----
                zt_mb = zpool.tile([P, RO, TB], bf16, tag="zt")

                def kw_stage(ro):
                    ps = [
                        pk_pool.tile([P, NC_CHUNK], f32, tag="pk", name=f"pk{c}") for c in range(CPB)
                    ]
                    for d in range(KO):
                        for c in range(CPB):
                            nc.tensor.matmul(
                                ps[c][:],
                                wk_sb[:, d, ro * P : (ro + 1) * P],
                                xtk_mb[:, d, c * NC_CHUNK : (c + 1) * NC_CHUNK],
                                start=(d == 0),
                                stop=(d == KO - 1),
                            )
                    st = stpool.tile([P, TB], f32, tag="st")
                    for c in range(CPB):
                        cs = slice(c * NC_CHUNK, (c + 1) * NC_CHUNK)
                        gs = slice(t0 + c * NC_CHUNK, t0 + (c + 1) * NC_CHUNK)
                        if SCAN_AS_COPY:
                            # timing-control variant: same data movement, no
                            # recurrence (WRONG math; perf probing only)
                            nc.vector.tensor_copy(st[:, cs], ps[c][:])
                        else:
                            nc.vector.tensor_tensor_scan(
                                st[:, cs],
                                ps[c][:],
                                invn_sb[:, gs],
                                (
                                    (0.0 if mb == 0 else carry_sb[:, ro : ro + 1])
                                    if c == 0
                                    else st[:, c * NC_CHUNK - 1 : c * NC_CHUNK]
                                ),
                                ADD,
                                BYP,
                            )
                    nc.vector.tensor_copy(
                        carry_sb[:, ro : ro + 1], st[:, TB - 1 : TB]
                    )
                    nc.vector.tensor_tensor(
                        zt_mb[:, ro, :], st[:], at_mb[:, ro, :], MUL
                    )

                if KW_FIRST:
                    for ro in range(RO):
                        kw_stage(ro)
                    for ro in range(RO):
                        a_stage(ro)
                else:
                    for ro in range(RO):
                        a_stage(ro)
                    for ro in range(RO):
                        kw_stage(ro)

                # ---- Y stage: Y^T[e,t] = (Z^T)^T-contraction with UW + ybias ----
                for eo in range(EO):
                    ps = [
                        py_pool.tile([P, NC_CHUNK], f32, tag="py", name=f"py{c}") for c in range(CPB)
                    ]
                    for r in range(RO):
                        for c in range(CPB):
                            nc.tensor.matmul(
                                ps[c][:],
                                wy_sb[:, r, eo * P : (eo + 1) * P],
                                zt_mb[:, r, c * NC_CHUNK : (c + 1) * NC_CHUNK],
                                start=(r == 0),
                                stop=(r == RO - 1),
                            )
                    for c in range(CPB):
                        ytile = ypool.tile([P, NC_CHUNK], f32, tag="yt")
                        nc.vector.tensor_scalar_add(
                            ytile[:], ps[c][:], ybias_sb[:, eo : eo + 1]
                        )
                        nc.sync.dma_start(
                            yt_r[
                                :,
                                eo,
                                t0 + c * NC_CHUNK : t0 + (c + 1) * NC_CHUNK,
                            ],
                            ytile[:],
                        )
    return nc


def _prepare(x, attention_mask, W_Q, W_K, U, V, Wb, bias, W_O):
    x = np.asarray(x, dtype=np.float32)
    attention_mask = np.asarray(attention_mask)
    W_Q = np.asarray(W_Q, dtype=np.float32)
    W_K = np.asarray(W_K, dtype=np.float32)
    U = np.asarray(U, dtype=np.float32)
    V = np.asarray(V, dtype=np.float32)
    Wb = np.asarray(Wb, dtype=np.float32)
    bias = np.asarray(bias, dtype=np.float32)
    W_O = np.asarray(W_O, dtype=np.float32)

    m = attention_mask.astype(np.float32)          # (B,T)
    all_ones = bool(np.all(m == 1.0))

    # Host-side weight folding (exact math, fp32).
    WQV = (W_Q.T @ V).astype(_bf16)                # (D,R)
    WKW = (W_K.T @ Wb).astype(_bf16)               # (D,R)
    UW = (W_O @ U).T.astype(_bf16)                 # (R,D)
    ybias_e = (W_O @ bias).astype(np.float32)      # (D,)
    ybias_tile = np.ascontiguousarray(
        ybias_e.reshape(EO, P).T
    )                                              # (P, EO) [ei, eo]

    n = np.cumsum(m, axis=1)                       # (B,T)
    invn = (1.0 / np.clip(n, 1.0, None)).astype(np.float32)

    nc = _build_program(dual_x=not all_ones)

    in_maps = []
    for b in range(B):
        xt_b = np.ascontiguousarray(x[b].T).astype(_bf16)     # (D,T)
        im = {
            "xt": xt_b,
            "wa": WQV,
            "wk": WKW,
            "wy": UW,
            "invn": np.ascontiguousarray(
                np.broadcast_to(invn[b][None, :], (P, T))
            ),
            "ybias": ybias_tile,
        }
        if not all_ones:
            im["xtk"] = np.ascontiguousarray(
                (x[b] * m[b][:, None]).T
            ).astype(_bf16)
        in_maps.append(im)

    return nc, in_maps


def _gather(res):
    out = np.empty((B, T, D), dtype=np.float32)
    for b in range(B):
        out[b] = res.results[b]["yt"].T
    return out


def kernel(x, attention_mask, W_Q, W_K, U, V, Wb, bias, W_O):
    from concourse.bass_utils import run_bass_kernel_spmd

    nc, in_maps = _prepare(x, attention_mask, W_Q, W_K, U, V, Wb, bias, W_O)
    res = run_bass_kernel_spmd(nc, in_maps, core_ids=list(range(B)))
    return _gather(res)


def run_timed(np_inputs, k1=16, k2=64, reps=5):
    """Estimate per-execution device time.

    This axon build has no NTFF profiling hook, so we measure the marginal
    wall-clock of asynchronously queued executions: dispatch K in flight,
    block once; slope between K=k1 and K=k2 amortizes the ~0.5 ms
    per-dispatch relay overhead. Returns ns per execution (upper bound on
    HW kernel time)."""
    import time

    import jax
    import jax.core
    from jax.sharding import Mesh, NamedSharding, PartitionSpec
    from jax.experimental.shard_map import shard_map

    import concourse.mybir as mybir
    from concourse import bass2jax

    nc, in_maps = _prepare(**np_inputs)
    bass2jax.install_neuronx_cc_hook()
    partition_name = nc.partition_id_tensor.name if nc.partition_id_tensor else None

    in_names, out_names, out_avals = [], [], []
    for alloc in nc.m.functions[0].allocations:
        if not isinstance(alloc, mybir.MemoryLocationSet):
            continue
        name = alloc.memorylocations[0].name
        if alloc.kind == "ExternalInput":
            if name != partition_name:
                in_names.append(name)
        elif alloc.kind == "ExternalOutput":
            out_names.append(name)
            out_avals.append(
                jax.core.ShapedArray(
                    tuple(alloc.tensor_shape), mybir.dt.np(alloc.dtype)
                )
            )
    all_names = in_names + out_names
    if partition_name is not None:
        all_names = all_names + [partition_name]

    def _body(*args):
        operands = list(args)
        if partition_name is not None:
            operands.append(bass2jax.partition_id_tensor())
        return tuple(
            bass2jax._bass_exec_p.bind(
                *operands,
                out_avals=tuple(out_avals),
                in_names=tuple(all_names),
                out_names=tuple(out_names),
                lowering_input_output_aliases=(),
                sim_require_finite=True,
                sim_require_nnan=True,
                nc=nc,
            )
        )

    devices = jax.devices()[:B]
    mesh = Mesh(np.array(devices), ("core",))
    nin = len(in_names) + len(out_names)
    sharded = jax.jit(
        shard_map(
            _body,
            mesh=mesh,
            in_specs=(PartitionSpec("core"),) * nin,
            out_specs=(PartitionSpec("core"),) * len(out_names),
            check_rep=False,
        ),
        keep_unused=True,
    )
    concat_in = [
        np.concatenate([np.asarray(in_maps[c][nm]) for c in range(B)], axis=0)
        for nm in in_names
    ]
    concat_zeros = [
        np.zeros((B * a.shape[0], *a.shape[1:]), a.dtype) for a in out_avals
    ]
    sharding = NamedSharding(mesh, PartitionSpec("core"))
    dev_args = [jax.device_put(a, sharding) for a in concat_in + concat_zeros]

    best = {}
    jax.block_until_ready(sharded(*dev_args))  # warm-up / compile
    for K in (k1, k2):
        b = None
        for _ in range(reps):
            t0 = time.perf_counter()
            outs = [sharded(*dev_args) for _ in range(K)]
            jax.block_until_ready(outs)
            dt = time.perf_counter() - t0
            b = dt if b is None else min(b, dt)
        best[K] = b
    return (best[k2] - best[k1]) / (k2 - k1) * 1e9


# revision 17
# speedup vs baseline: 1.0069x; 1.0069x over previous
"""Trainium2 Bass kernel for CausalBCNAttention.

Reference math (per batch b):
    Q = x @ W_Q^T ; K = x @ W_K^T
    A = Q @ V ; Kw = (K @ Wb) * mask
    S = cumsum_t(Kw) ; n = cumsum_t(mask)
    out = ((A*S) @ U^T + n*bias) / max(n,1)
    Y = out @ W_O^T

Algebraic refactoring (exact up to fp reassociation):
    A  = x @ (W_Q^T V)            -- WQV precomputed host-side
    Kw = (x*mask) @ (W_K^T Wb)    -- WKW precomputed host-side
    S  = cumsum_t(Kw)
    Y  = ((A/n) (.) S) @ (U^T W_O^T) + bias @ W_O^T
because the /n factor is a per-row scalar and commutes through right-matmuls.
So the device does 3 GEMMs of (4096x1024)@(1024x1024) per batch plus a
prefix-scan, instead of 6 GEMMs.

Device mapping: data-parallel over B across the 8 NeuronCores (core b owns
batch b). All tensors live "transposed" ([feature, t] with t on the free
dimension): the GEMM chain then needs no on-chip transposes and the causal
cumsum is a native DVE prefix scan (tensor_tensor_scan) along the free dim.
Matmuls run in bf16 with fp32 PSUM accumulation; the scan state is fp32.
"""

import numpy as np
import ml_dtypes

B, T, D, R = 8, 4096, 1024, 1024
P = 128
KO = D // P          # 8 contraction subtiles
RO = R // P          # 8 r tiles
EO = D // P          # 8 output-feature tiles
TB = 1024            # t megablock
NB = T // TB         # 4 megablocks
NC_CHUNK = 512       # matmul free-dim chunk (one PSUM bank)
CPB = TB // NC_CHUNK  # chunks per megablock

_bf16 = ml_dtypes.bfloat16

# Perf-experiment knobs (perf tooling overrides these module globals; the
# graded path always uses the defaults).
PSUM_BUFS = (2, 3, 3)
SCAN_AS_COPY = False
KW_FIRST = False
ST_BUFS = 2
INTERLEAVE_AKW = False
Y_ON_ACT = False
Z_ON_POOL = False


def _patch_tile_drain():
    """This container's walrus codegen rejects any instruction carrying more
    than one sync wait ("Too many sync wait commands"). Two fixes:
    (1) TileContext._add_instruction: hoist N>1 scheduler-attached waits onto
        N standalone NoOps emitted just before the instruction on the same
        engine stream (identical semantics: waits happen-before the op).
    (2) TileContext._drain_and_barrier: the tail drain aggregates waits for
        every active proc onto one Drain; split into one Drain per wait."""
    import bass_rust
    import concourse.mybir as mybir
    import concourse.tile as tile
    from concourse.vector_clock import ScopedClock

    if getattr(tile.TileContext, "_waitsplit_patched", False):
        return

    counter = [0]

    def _split_waits_add_instruction(self, inst):
        si = inst.sync_info
        if si is not None and len(si.on_wait) > 1:
            waits = list(si.on_wait)
            for w in waits:
                counter[0] += 1
                nop = mybir.InstNoOp(name=f"waitsplit-{counter[0]}")
                nop.engine = inst.engine
                nop.sync_info = bass_rust.SyncInfo(on_wait=[w], on_update=[])
                self.nc.register_instruction(nop, overwrite=True)
                self.nc.cur_bb.bb.add_instruction(nop)
            si.on_wait = []
            inst.sync_info = si
        self.nc.register_instruction(inst, overwrite=True)
        self.nc.cur_bb.bb.add_instruction(inst)

    def _split_drain_and_barrier(self, tick_clock, wait_clock):
        nc = self.nc
        drain_inst = nc.sync.drain()
        wait_clock.add_sem_waits(
            drain_inst.ins, ScopedClock({None: tick_clock.global_clock})
        )
        si = drain_inst.ins.sync_info
        waits = list(si.on_wait) if si is not None else []
        if len(waits) > 1:
            si.on_wait = waits[:1]
            drain_inst.ins.sync_info = si
            for w in waits[1:]:
                extra = nc.sync.drain()
                extra.ins.sync_info = bass_rust.SyncInfo(on_wait=[w], on_update=[])

        nc.all_engine_barrier()
        assert self.sems is not None
        popped = nc._tile_sem_poison_stack.pop()
        assert popped is self._sem_poison
        nc.clear_and_free_semaphores(list(self.sems.allocated().values()))
        nc.all_engine_barrier()

    tile.TileContext._add_instruction = _split_waits_add_instruction
    tile.TileContext._drain_and_barrier = _split_drain_and_barrier
    tile.TileContext._waitsplit_patched = True


def _build_program(dual_x: bool):
    """Trace the Bass/Tile program (identical on every core; SPMD)."""
    import concourse.bass as bass
    import concourse.mybir as mybir
    import concourse.tile as tile

    _patch_tile_drain()
    f32 = mybir.dt.float32
    bf16 = mybir.dt.bfloat16
    MUL = mybir.AluOpType.mult
    ADD = mybir.AluOpType.add
    BYP = mybir.AluOpType.bypass
    COPYF = mybir.ActivationFunctionType.Identity

    nc = bass.Bass()
    xt = nc.dram_tensor("xt", [D, T], bf16, kind="ExternalInput")
    xtk = (
        nc.dram_tensor("xtk", [D, T], bf16, kind="ExternalInput") if dual_x else xt
    )
    wa = nc.dram_tensor("wa", [D, R], bf16, kind="ExternalInput")
    wk = nc.dram_tensor("wk", [D, R], bf16, kind="ExternalInput")
    wy = nc.dram_tensor("wy", [R, D], bf16, kind="ExternalInput")
    invn = nc.dram_tensor("invn", [P, T], f32, kind="ExternalInput")
    ybias = nc.dram_tensor("ybias", [P, EO], f32, kind="ExternalInput")
    yt = nc.dram_tensor("yt", [D, T], f32, kind="ExternalOutput")

    xt_r = xt.rearrange("(ko ki) t -> ki ko t", ki=P)
    xtk_r = xtk.rearrange("(ko ki) t -> ki ko t", ki=P)
    wa_r = wa.rearrange("(ko ki) r -> ki ko r", ki=P)
    wk_r = wk.rearrange("(ko ki) r -> ki ko r", ki=P)
    wy_r = wy.rearrange("(ro ri) e -> ri ro e", ri=P)
    yt_r = yt.rearrange("(eo ei) t -> ei eo t", ei=P)

    with tile.TileContext(nc) as tc:
        with (
            tc.tile_pool(name="weights", bufs=1) as wpool,
            tc.tile_pool(name="consts", bufs=1) as cpool,
            tc.tile_pool(name="xin", bufs=2) as xpool,
            tc.tile_pool(name="xkin", bufs=2 if dual_x else 1) as xkpool,
            # dual_x adds 32KB/partition of xtk tiles; shrink the A' pool to
            # stay under the 192KB/partition SBUF budget on that (cold) path.
            tc.tile_pool(name="aprime", bufs=1 if dual_x else 2) as apool,
            tc.tile_pool(name="zbuf", bufs=2) as zpool,
            tc.tile_pool(name="stbuf", bufs=ST_BUFS) as stpool,
            tc.tile_pool(name="carry", bufs=1) as crpool,
            tc.tile_pool(name="yout", bufs=2 if dual_x else 4) as ypool,
            tc.tile_pool(name="pa", bufs=PSUM_BUFS[0], space="PSUM") as pa_pool,
            tc.tile_pool(name="pk", bufs=PSUM_BUFS[1], space="PSUM") as pk_pool,
            tc.tile_pool(name="py", bufs=PSUM_BUFS[2], space="PSUM") as py_pool,
        ):
            # Preamble loads are sliced per contraction subtile so the first
            # matmuls (which need only wa[:, 0] + xt0[:, 0]) can start after
            # <1 MB of DMA instead of after every weight has landed.
            wa_sb = wpool.tile([P, KO, R], bf16, tag="wa")
            wk_sb = wpool.tile([P, KO, R], bf16, tag="wk")
            wy_sb = wpool.tile([P, RO, D], bf16, tag="wy")
            xt0 = xpool.tile([P, KO, TB], bf16, tag="xt", name="xt0")
            invn_sb = cpool.tile([P, T], f32, tag="invn")
            for d in range(KO):
                nc.sync.dma_start(wa_sb[:, d], wa_r[:, d])
                nc.sync.dma_start(xt0[:, d], xt_r[:, d, 0:TB])
                nc.sync.dma_start(wk_sb[:, d], wk_r[:, d])
                nc.sync.dma_start(
                    invn_sb[:, d * (T // KO) : (d + 1) * (T // KO)],
                    invn[:, d * (T // KO) : (d + 1) * (T // KO)],
                )
            nc.sync.dma_start(wy_sb[:], wy_r[:])
            ybias_sb = cpool.tile([P, EO], f32, tag="ybias")
            nc.sync.dma_start(ybias_sb[:], ybias[:])
            carry_sb = crpool.tile([P, RO], f32, tag="carry")

            for mb in range(NB):
                t0 = mb * TB

                if mb == 0:
                    xt_mb = xt0
                else:
                    xt_mb = xpool.tile([P, KO, TB], bf16, tag="xt")
                    nc.sync.dma_start(xt_mb[:], xt_r[:, :, t0 : t0 + TB])
                if dual_x:
                    xtk_mb = xkpool.tile([P, KO, TB], bf16, tag="xtk")
                    nc.sync.dma_start(xtk_mb[:], xtk_r[:, :, t0 : t0 + TB])
                else:
                    xtk_mb = xt_mb

                # ---- A' stage: A'[r,t] = (x @ WQV)^T[r,t] * invn[t] ----
                at_mb = apool.tile([P, RO, TB], bf16, tag="at")

                def a_stage(ro):
                    ps = [
                        pa_pool.tile([P, NC_CHUNK], f32, tag="pa", name=f"pa{c}") for c in range(CPB)
                    ]
                    for d in range(KO):
                        for c in range(CPB):
                            nc.tensor.matmul(
                                ps[c][:],
                                wa_sb[:, d, ro * P : (ro + 1) * P],
                                xt_mb[:, d, c * NC_CHUNK : (c + 1) * NC_CHUNK],
                                start=(d == 0),
                                stop=(d == KO - 1),
                            )
                    for c in range(CPB):
                        cs = slice(c * NC_CHUNK, (c + 1) * NC_CHUNK)
                        gs = slice(t0 + c * NC_CHUNK, t0 + (c + 1) * NC_CHUNK)
                        nc.vector.tensor_tensor(
                            at_mb[:, ro, cs], ps[c][:], invn_sb[:, gs], MUL
                        )

                # ---- Kw stage + causal prefix scan + Z ----
                zt_mb = zpool.tile([P, RO, TB], bf16, tag="zt")

                def kw_stage(ro):
                    ps = [
                        pk_pool.tile([P, NC_CHUNK], f32, tag="pk", name=f"pk{c}") for c in range(CPB)
                    ]
                    for d in range(KO):
                        for c in range(CPB):
                            nc.tensor.matmul(
                                ps[c][:],
                                wk_sb[:, d, ro * P : (ro + 1) * P],
                                xtk_mb[:, d, c * NC_CHUNK : (c + 1) * NC_CHUNK],
                                start=(d == 0),
                                stop=(d == KO - 1),
                            )
                    st = stpool.tile([P, TB], f32, tag="st")
                    for c in range(CPB):
                        cs = slice(c * NC_CHUNK, (c + 1) * NC_CHUNK)
                        gs = slice(t0 + c * NC_CHUNK, t0 + (c + 1) * NC_CHUNK)
                        if SCAN_AS_COPY:
                            # timing-control variant: same data movement, no
                            # recurrence (WRONG math; perf probing only)
                            nc.vector.tensor_copy(st[:, cs], ps[c][:])
                        else:
                            nc.vector.tensor_tensor_scan(
                                st[:, cs],
                                ps[c][:],
                                invn_sb[:, gs],
                                (
                                    (0.0 if mb == 0 else carry_sb[:, ro : ro + 1])
                                    if c == 0
                                    else st[:, c * NC_CHUNK - 1 : c * NC_CHUNK]
                                ),
                                ADD,
                                BYP,
                            )
                    nc.vector.tensor_copy(
                        carry_sb[:, ro : ro + 1], st[:, TB - 1 : TB]
                    )
                    if Z_ON_POOL:
                        nc.gpsimd.tensor_tensor(
                            zt_mb[:, ro, :], st[:], at_mb[:, ro, :], MUL
                        )
                    else:
                        nc.vector.tensor_tensor(
                            zt_mb[:, ro, :], st[:], at_mb[:, ro, :], MUL
                        )

                if INTERLEAVE_AKW:
                    for ro in range(RO):
                        a_stage(ro)
                        kw_stage(ro)
                elif KW_FIRST:
                    for ro in range(RO):
                        kw_stage(ro)
                    for ro in range(RO):
                        a_stage(ro)
                else:
                    for ro in range(RO):
                        a_stage(ro)
                    for ro in range(RO):
                        kw_stage(ro)

                # ---- Y stage: Y^T[e,t] = (Z^T)^T-contraction with UW + ybias ----
                for eo in range(EO):
                    ps = [
                        py_pool.tile([P, NC_CHUNK], f32, tag="py", name=f"py{c}") for c in range(CPB)
                    ]
                    for r in range(RO):
                        for c in range(CPB):
                            nc.tensor.matmul(
                                ps[c][:],
                                wy_sb[:, r, eo * P : (eo + 1) * P],
                                zt_mb[:, r, c * NC_CHUNK : (c + 1) * NC_CHUNK],
                                start=(r == 0),
                                stop=(r == RO - 1),
                            )
                    for c in range(CPB):
                        ytile = ypool.tile([P, NC_CHUNK], f32, tag="yt")
                        if Y_ON_ACT:
                            nc.scalar.activation(
                                ytile[:], ps[c][:], COPYF,
                                bias=ybias_sb[:, eo : eo + 1],
                            )
                        else:
                            nc.vector.tensor_scalar_add(
                                ytile[:], ps[c][:], ybias_sb[:, eo : eo + 1]
                            )
                        nc.sync.dma_start(
                            yt_r[
                                :,
                                eo,
                                t0 + c * NC_CHUNK : t0 + (c + 1) * NC_CHUNK,
                            ],
                            ytile[:],
                        )
    return nc


def _prepare(x, attention_mask, W_Q, W_K, U, V, Wb, bias, W_O):
    x = np.asarray(x, dtype=np.float32)
    attention_mask = np.asarray(attention_mask)
    W_Q = np.asarray(W_Q, dtype=np.float32)
    W_K = np.asarray(W_K, dtype=np.float32)
    U = np.asarray(U, dtype=np.float32)
    V = np.asarray(V, dtype=np.float32)
    Wb = np.asarray(Wb, dtype=np.float32)
    bias = np.asarray(bias, dtype=np.float32)
    W_O = np.asarray(W_O, dtype=np.float32)

    m = attention_mask.astype(np.float32)          # (B,T)
    all_ones = bool(np.all(m == 1.0))

    # Host-side weight folding (exact math, fp32).
    WQV = (W_Q.T @ V).astype(_bf16)                # (D,R)
    WKW = (W_K.T @ Wb).astype(_bf16)               # (D,R)
    UW = (W_O @ U).T.astype(_bf16)                 # (R,D)
    ybias_e = (W_O @ bias).astype(np.float32)      # (D,)
    ybias_tile = np.ascontiguousarray(
        ybias_e.reshape(EO, P).T
    )                                              # (P, EO) [ei, eo]

    n = np.cumsum(m, axis=1)                       # (B,T)
    invn = (1.0 / np.clip(n, 1.0, None)).astype(np.float32)

    nc = _build_program(dual_x=not all_ones)

    in_maps = []
    for b in range(B):
        xt_b = np.ascontiguousarray(x[b].T).astype(_bf16)     # (D,T)
        im = {
            "xt": xt_b,
            "wa": WQV,
            "wk": WKW,
            "wy": UW,
            "invn": np.ascontiguousarray(
                np.broadcast_to(invn[b][None, :], (P, T))
            ),
            "ybias": ybias_tile,
        }
        if not all_ones:
            im["xtk"] = np.ascontiguousarray(
                (x[b] * m[b][:, None]).T
            ).astype(_bf16)
        in_maps.append(im)

    return nc, in_maps


def _gather(res):
    out = np.empty((B, T, D), dtype=np.float32)
    for b in range(B):
        out[b] = res.results[b]["yt"].T
    return out


def kernel(x, attention_mask, W_Q, W_K, U, V, Wb, bias, W_O):
    from concourse.bass_utils import run_bass_kernel_spmd

    nc, in_maps = _prepare(x, attention_mask, W_Q, W_K, U, V, Wb, bias, W_O)
    res = run_bass_kernel_spmd(nc, in_maps, core_ids=list(range(B)))
    return _gather(res)


def run_timed(np_inputs, k1=16, k2=64, reps=5):
    """Estimate per-execution device time.

    This axon build has no NTFF profiling hook, so we measure the marginal
    wall-clock of asynchronously queued executions: dispatch K in flight,
    block once; slope between K=k1 and K=k2 amortizes the ~0.5 ms
    per-dispatch relay overhead. Returns ns per execution (upper bound on
    HW kernel time)."""
    import time

    import jax
    import jax.core
    from jax.sharding import Mesh, NamedSharding, PartitionSpec
    from jax.experimental.shard_map import shard_map

    import concourse.mybir as mybir
    from concourse import bass2jax

    nc, in_maps = _prepare(**np_inputs)
    bass2jax.install_neuronx_cc_hook()
    partition_name = nc.partition_id_tensor.name if nc.partition_id_tensor else None

    in_names, out_names, out_avals = [], [], []
    for alloc in nc.m.functions[0].allocations:
        if not isinstance(alloc, mybir.MemoryLocationSet):
            continue
        name = alloc.memorylocations[0].name
        if alloc.kind == "ExternalInput":
            if name != partition_name:
                in_names.append(name)
        elif alloc.kind == "ExternalOutput":
            out_names.append(name)
            out_avals.append(
                jax.core.ShapedArray(
                    tuple(alloc.tensor_shape), mybir.dt.np(alloc.dtype)
                )
            )
    all_names = in_names + out_names
    if partition_name is not None:
        all_names = all_names + [partition_name]

    def _body(*args):
        operands = list(args)
        if partition_name is not None:
            operands.append(bass2jax.partition_id_tensor())
        return tuple(
            bass2jax._bass_exec_p.bind(
                *operands,
                out_avals=tuple(out_avals),
                in_names=tuple(all_names),
                out_names=tuple(out_names),
                lowering_input_output_aliases=(),
                sim_require_finite=True,
                sim_require_nnan=True,
                nc=nc,
            )
        )

    devices = jax.devices()[:B]
    mesh = Mesh(np.array(devices), ("core",))
    nin = len(in_names) + len(out_names)
    sharded = jax.jit(
        shard_map(
            _body,
            mesh=mesh,
            in_specs=(PartitionSpec("core"),) * nin,
            out_specs=(PartitionSpec("core"),) * len(out_names),
            check_rep=False,
        ),
        keep_unused=True,
    )
    concat_in = [
        np.concatenate([np.asarray(in_maps[c][nm]) for c in range(B)], axis=0)
        for nm in in_names
    ]
    concat_zeros = [
        np.zeros((B * a.shape[0], *a.shape[1:]), a.dtype) for a in out_avals
    ]
    sharding = NamedSharding(mesh, PartitionSpec("core"))
    dev_args = [jax.device_put(a, sharding) for a in concat_in + concat_zeros]

    best = {}
    jax.block_until_ready(sharded(*dev_args))  # warm-up / compile
    for K in (k1, k2):
        b = None
        for _ in range(reps):
            t0 = time.perf_counter()
            outs = [sharded(*dev_args) for _ in range(K)]
            jax.block_until_ready(outs)
            dt = time.perf_counter() - t0
            b = dt if b is None else min(b, dt)
        best[K] = b
    return (best[k2] - best[k1]) / (k2 - k1) * 1e9


# revision 19
# speedup vs baseline: 1.3507x; 1.3415x over previous
"""Trainium2 Bass kernel for CausalBCNAttention.

Reference math (per batch b):
    Q = x @ W_Q^T ; K = x @ W_K^T
    A = Q @ V ; Kw = (K @ Wb) * mask
    S = cumsum_t(Kw) ; n = cumsum_t(mask)
    out = ((A*S) @ U^T + n*bias) / max(n,1)
    Y = out @ W_O^T

Algebraic refactoring (exact up to fp reassociation):
    A  = x @ (W_Q^T V)            -- WQV precomputed host-side
    Kw = (x*mask) @ (W_K^T Wb)    -- WKW precomputed host-side
    S  = cumsum_t(Kw)
    Y  = ((A/n) (.) S) @ (U^T W_O^T) + bias @ W_O^T
because the /n factor is a per-row scalar and commutes through right-matmuls.
So the device does 3 GEMMs of (4096x1024)@(1024x1024) per batch plus a
prefix-scan, instead of 6 GEMMs.

Device mapping: data-parallel over B across the 8 NeuronCores (core b owns
batch b). All tensors live "transposed" ([feature, t] with t on the free
dimension): the GEMM chain then needs no on-chip transposes and the causal
cumsum is a native DVE prefix scan (tensor_tensor_scan) along the free dim.
Matmuls run in bf16 with fp32 PSUM accumulation; the scan state is fp32.
"""

import numpy as np
import ml_dtypes

B, T, D, R = 8, 4096, 1024, 1024
P = 128
KO = D // P          # 8 contraction subtiles
RO = R // P          # 8 r tiles
EO = D // P          # 8 output-feature tiles
TB = 1024            # t megablock
NB = T // TB         # 4 megablocks
NC_CHUNK = 512       # matmul free-dim chunk (one PSUM bank)
CPB = TB // NC_CHUNK  # chunks per megablock

_bf16 = ml_dtypes.bfloat16

# Perf-experiment knobs (perf tooling overrides these module globals; the
# graded path always uses the defaults).
PSUM_BUFS = (8, 1, 1)
SCAN_AS_COPY = False
KW_FIRST = False
ST_BUFS = 2
INTERLEAVE_AKW = False
Y_ON_ACT = False
Z_ON_POOL = False
SHARED_PSUM = True


def _patch_tile_drain():
    """This container's walrus codegen rejects any instruction carrying more
    than one sync wait ("Too many sync wait commands"). Two fixes:
    (1) TileContext._add_instruction: hoist N>1 scheduler-attached waits onto
        N standalone NoOps emitted just before the instruction on the same
        engine stream (identical semantics: waits happen-before the op).
    (2) TileContext._drain_and_barrier: the tail drain aggregates waits for
        every active proc onto one Drain; split into one Drain per wait."""
    import bass_rust
    import concourse.mybir as mybir
    import concourse.tile as tile
    from concourse.vector_clock import ScopedClock

    if getattr(tile.TileContext, "_waitsplit_patched", False):
        return

    counter = [0]

    def _split_waits_add_instruction(self, inst):
        si = inst.sync_info
        if si is not None and len(si.on_wait) > 1:
            waits = list(si.on_wait)
            for w in waits:
                counter[0] += 1
                nop = mybir.InstNoOp(name=f"waitsplit-{counter[0]}")
                nop.engine = inst.engine
                nop.sync_info = bass_rust.SyncInfo(on_wait=[w], on_update=[])
                self.nc.register_instruction(nop, overwrite=True)
                self.nc.cur_bb.bb.add_instruction(nop)
            si.on_wait = []
            inst.sync_info = si
        self.nc.register_instruction(inst, overwrite=True)
        self.nc.cur_bb.bb.add_instruction(inst)

    def _split_drain_and_barrier(self, tick_clock, wait_clock):
        nc = self.nc
        drain_inst = nc.sync.drain()
        wait_clock.add_sem_waits(
            drain_inst.ins, ScopedClock({None: tick_clock.global_clock})
        )
        si = drain_inst.ins.sync_info
        waits = list(si.on_wait) if si is not None else []
        if len(waits) > 1:
            si.on_wait = waits[:1]
            drain_inst.ins.sync_info = si
            for w in waits[1:]:
                extra = nc.sync.drain()
                extra.ins.sync_info = bass_rust.SyncInfo(on_wait=[w], on_update=[])

        nc.all_engine_barrier()
        assert self.sems is not None
        popped = nc._tile_sem_poison_stack.pop()
        assert popped is self._sem_poison
        nc.clear_and_free_semaphores(list(self.sems.allocated().values()))
        nc.all_engine_barrier()

    tile.TileContext._add_instruction = _split_waits_add_instruction
    tile.TileContext._drain_and_barrier = _split_drain_and_barrier
    tile.TileContext._waitsplit_patched = True


def _build_program(dual_x: bool):
    """Trace the Bass/Tile program (identical on every core; SPMD)."""
    import concourse.bass as bass
    import concourse.mybir as mybir
    import concourse.tile as tile

    _patch_tile_drain()
    f32 = mybir.dt.float32
    bf16 = mybir.dt.bfloat16
    MUL = mybir.AluOpType.mult
    ADD = mybir.AluOpType.add
    BYP = mybir.AluOpType.bypass
    COPYF = mybir.ActivationFunctionType.Identity

    nc = bass.Bass()
    xt = nc.dram_tensor("xt", [D, T], bf16, kind="ExternalInput")
    xtk = (
        nc.dram_tensor("xtk", [D, T], bf16, kind="ExternalInput") if dual_x else xt
    )
    wa = nc.dram_tensor("wa", [D, R], bf16, kind="ExternalInput")
    wk = nc.dram_tensor("wk", [D, R], bf16, kind="ExternalInput")
    wy = nc.dram_tensor("wy", [R, D], bf16, kind="ExternalInput")
    invn = nc.dram_tensor("invn", [P, T], f32, kind="ExternalInput")
    ybias = nc.dram_tensor("ybias", [P, EO], f32, kind="ExternalInput")
    yt = nc.dram_tensor("yt", [D, T], f32, kind="ExternalOutput")

    xt_r = xt.rearrange("(ko ki) t -> ki ko t", ki=P)
    xtk_r = xtk.rearrange("(ko ki) t -> ki ko t", ki=P)
    wa_r = wa.rearrange("(ko ki) r -> ki ko r", ki=P)
    wk_r = wk.rearrange("(ko ki) r -> ki ko r", ki=P)
    wy_r = wy.rearrange("(ro ri) e -> ri ro e", ri=P)
    yt_r = yt.rearrange("(eo ei) t -> ei eo t", ei=P)

    with tile.TileContext(nc) as tc:
        with (
            tc.tile_pool(name="weights", bufs=1) as wpool,
            tc.tile_pool(name="consts", bufs=1) as cpool,
            tc.tile_pool(name="xin", bufs=2) as xpool,
            tc.tile_pool(name="xkin", bufs=2 if dual_x else 1) as xkpool,
            # dual_x adds 32KB/partition of xtk tiles; shrink the A' pool to
            # stay under the 192KB/partition SBUF budget on that (cold) path.
            tc.tile_pool(name="aprime", bufs=1 if dual_x else 2) as apool,
            tc.tile_pool(name="zbuf", bufs=2) as zpool,
            tc.tile_pool(name="stbuf", bufs=ST_BUFS) as stpool,
            tc.tile_pool(name="carry", bufs=1) as crpool,
            tc.tile_pool(name="yout", bufs=2 if dual_x else 4) as ypool,
            tc.tile_pool(name="pa", bufs=PSUM_BUFS[0], space="PSUM") as pa_pool,
            tc.tile_pool(name="pk", bufs=PSUM_BUFS[1], space="PSUM") as pk_pool,
            tc.tile_pool(name="py", bufs=PSUM_BUFS[2], space="PSUM") as py_pool,
        ):
            if SHARED_PSUM:
                pk_pool = pa_pool
                py_pool = pa_pool
            # Preamble loads are sliced per contraction subtile so the first
            # matmuls (which need only wa[:, 0] + xt0[:, 0]) can start after
            # <1 MB of DMA instead of after every weight has landed.
            wa_sb = wpool.tile([P, KO, R], bf16, tag="wa")
            wk_sb = wpool.tile([P, KO, R], bf16, tag="wk")
            wy_sb = wpool.tile([P, RO, D], bf16, tag="wy")
            xt0 = xpool.tile([P, KO, TB], bf16, tag="xt", name="xt0")
            invn_sb = cpool.tile([P, T], f32, tag="invn")
            for d in range(KO):
                nc.sync.dma_start(wa_sb[:, d], wa_r[:, d])
                nc.sync.dma_start(xt0[:, d], xt_r[:, d, 0:TB])
                nc.sync.dma_start(wk_sb[:, d], wk_r[:, d])
                nc.sync.dma_start(
                    invn_sb[:, d * (T // KO) : (d + 1) * (T // KO)],
                    invn[:, d * (T // KO) : (d + 1) * (T // KO)],
                )
            nc.sync.dma_start(wy_sb[:], wy_r[:])
            ybias_sb = cpool.tile([P, EO], f32, tag="ybias")
            nc.sync.dma_start(ybias_sb[:], ybias[:])
            carry_sb = crpool.tile([P, RO], f32, tag="carry")

            for mb in range(NB):
                t0 = mb * TB

                if mb == 0:
                    xt_mb = xt0
                else:
                    xt_mb = xpool.tile([P, KO, TB], bf16, tag="xt")
                    nc.sync.dma_start(xt_mb[:], xt_r[:, :, t0 : t0 + TB])
                if dual_x:
                    xtk_mb = xkpool.tile([P, KO, TB], bf16, tag="xtk")
                    nc.sync.dma_start(xtk_mb[:], xtk_r[:, :, t0 : t0 + TB])
                else:
                    xtk_mb = xt_mb

                # ---- A' stage: A'[r,t] = (x @ WQV)^T[r,t] * invn[t] ----
                at_mb = apool.tile([P, RO, TB], bf16, tag="at")

                def a_stage(ro):
                    ps = [
                        pa_pool.tile([P, NC_CHUNK], f32, tag="ps" if SHARED_PSUM else "pa", name=f"pa{c}") for c in range(CPB)
                    ]
                    for d in range(KO):
                        for c in range(CPB):
                            nc.tensor.matmul(
                                ps[c][:],
                                wa_sb[:, d, ro * P : (ro + 1) * P],
                                xt_mb[:, d, c * NC_CHUNK : (c + 1) * NC_CHUNK],
                                start=(d == 0),
                                stop=(d == KO - 1),
                            )
                    for c in range(CPB):
                        cs = slice(c * NC_CHUNK, (c + 1) * NC_CHUNK)
                        gs = slice(t0 + c * NC_CHUNK, t0 + (c + 1) * NC_CHUNK)
                        nc.vector.tensor_tensor(
                            at_mb[:, ro, cs], ps[c][:], invn_sb[:, gs], MUL
                        )

                # ---- Kw stage + causal prefix scan + Z ----
                zt_mb = zpool.tile([P, RO, TB], bf16, tag="zt")

                def kw_stage(ro):
                    ps = [
                        pk_pool.tile([P, NC_CHUNK], f32, tag="ps" if SHARED_PSUM else "pk", name=f"pk{c}") for c in range(CPB)
                    ]
                    for d in range(KO):
                        for c in range(CPB):
                            nc.tensor.matmul(
                                ps[c][:],
                                wk_sb[:, d, ro * P : (ro + 1) * P],
                                xtk_mb[:, d, c * NC_CHUNK : (c + 1) * NC_CHUNK],
                                start=(d == 0),
                                stop=(d == KO - 1),
                            )
                    st = stpool.tile([P, TB], f32, tag="st")
                    for c in range(CPB):
                        cs = slice(c * NC_CHUNK, (c + 1) * NC_CHUNK)
                        gs = slice(t0 + c * NC_CHUNK, t0 + (c + 1) * NC_CHUNK)
                        if SCAN_AS_COPY:
                            # timing-control variant: same data movement, no
                            # recurrence (WRONG math; perf probing only)
                            nc.vector.tensor_copy(st[:, cs], ps[c][:])
                        else:
                            nc.vector.tensor_tensor_scan(
                                st[:, cs],
                                ps[c][:],
                                invn_sb[:, gs],
                                (
                                    (0.0 if mb == 0 else carry_sb[:, ro : ro + 1])
                                    if c == 0
                                    else st[:, c * NC_CHUNK - 1 : c * NC_CHUNK]
                                ),
                                ADD,
                                BYP,
                            )
                    nc.vector.tensor_copy(
                        carry_sb[:, ro : ro + 1], st[:, TB - 1 : TB]
                    )
                    if Z_ON_POOL:
                        nc.gpsimd.tensor_tensor(
                            zt_mb[:, ro, :], st[:], at_mb[:, ro, :], MUL
                        )
                    else:
                        nc.vector.tensor_tensor(
                            zt_mb[:, ro, :], st[:], at_mb[:, ro, :], MUL
                        )

                if INTERLEAVE_AKW:
                    for ro in range(RO):
                        a_stage(ro)
                        kw_stage(ro)
                elif KW_FIRST:
                    for ro in range(RO):
                        kw_stage(ro)
                    for ro in range(RO):
                        a_stage(ro)
                else:
                    for ro in range(RO):
                        a_stage(ro)
                    for ro in range(RO):
                        kw_stage(ro)

                # ---- Y stage: Y^T[e,t] = (Z^T)^T-contraction with UW + ybias ----
                for eo in range(EO):
                    ps = [
                        py_pool.tile([P, NC_CHUNK], f32, tag="ps" if SHARED_PSUM else "py", name=f"py{c}") for c in range(CPB)
                    ]
                    for r in range(RO):
                        for c in range(CPB):
                            nc.tensor.matmul(
                                ps[c][:],
                                wy_sb[:, r, eo * P : (eo + 1) * P],
                                zt_mb[:, r, c * NC_CHUNK : (c + 1) * NC_CHUNK],
                                start=(r == 0),
                                stop=(r == RO - 1),
                            )
                    for c in range(CPB):
                        ytile = ypool.tile([P, NC_CHUNK], f32, tag="yt")
                        if Y_ON_ACT:
                            nc.scalar.activation(
                                ytile[:], ps[c][:], COPYF,
                                bias=ybias_sb[:, eo : eo + 1],
                            )
                        else:
                            nc.vector.tensor_scalar_add(
                                ytile[:], ps[c][:], ybias_sb[:, eo : eo + 1]
                            )
                        nc.sync.dma_start(
                            yt_r[
                                :,
                                eo,
                                t0 + c * NC_CHUNK : t0 + (c + 1) * NC_CHUNK,
                            ],
                            ytile[:],
                        )
    return nc


def _prepare(x, attention_mask, W_Q, W_K, U, V, Wb, bias, W_O):
    x = np.asarray(x, dtype=np.float32)
    attention_mask = np.asarray(attention_mask)
    W_Q = np.asarray(W_Q, dtype=np.float32)
    W_K = np.asarray(W_K, dtype=np.float32)
    U = np.asarray(U, dtype=np.float32)
    V = np.asarray(V, dtype=np.float32)
    Wb = np.asarray(Wb, dtype=np.float32)
    bias = np.asarray(bias, dtype=np.float32)
    W_O = np.asarray(W_O, dtype=np.float32)

    m = attention_mask.astype(np.float32)          # (B,T)
    all_ones = bool(np.all(m == 1.0))

    # Host-side weight folding (exact math, fp32).
    WQV = (W_Q.T @ V).astype(_bf16)                # (D,R)
    WKW = (W_K.T @ Wb).astype(_bf16)               # (D,R)
    UW = (W_O @ U).T.astype(_bf16)                 # (R,D)
    ybias_e = (W_O @ bias).astype(np.float32)      # (D,)
    ybias_tile = np.ascontiguousarray(
        ybias_e.reshape(EO, P).T
    )                                              # (P, EO) [ei, eo]

    n = np.cumsum(m, axis=1)                       # (B,T)
    invn = (1.0 / np.clip(n, 1.0, None)).astype(np.float32)

    nc = _build_program(dual_x=not all_ones)

    in_maps = []
    for b in range(B):
        xt_b = np.ascontiguousarray(x[b].T).astype(_bf16)     # (D,T)
        im = {
            "xt": xt_b,
            "wa": WQV,
            "wk": WKW,
            "wy": UW,
            "invn": np.ascontiguousarray(
                np.broadcast_to(invn[b][None, :], (P, T))
            ),
            "ybias": ybias_tile,
        }
        if not all_ones:
            im["xtk"] = np.ascontiguousarray(
                (x[b] * m[b][:, None]).T
            ).astype(_bf16)
        in_maps.append(im)

    return nc, in_maps


def _gather(res):
    out = np.empty((B, T, D), dtype=np.float32)
    for b in range(B):
        out[b] = res.results[b]["yt"].T
    return out


def kernel(x, attention_mask, W_Q, W_K, U, V, Wb, bias, W_O):
    from concourse.bass_utils import run_bass_kernel_spmd

    nc, in_maps = _prepare(x, attention_mask, W_Q, W_K, U, V, Wb, bias, W_O)
    res = run_bass_kernel_spmd(nc, in_maps, core_ids=list(range(B)))
    return _gather(res)


def run_timed(np_inputs, k1=16, k2=64, reps=5):
    """Estimate per-execution device time.

    This axon build has no NTFF profiling hook, so we measure the marginal
    wall-clock of asynchronously queued executions: dispatch K in flight,
    block once; slope between K=k1 and K=k2 amortizes the ~0.5 ms
    per-dispatch relay overhead. Returns ns per execution (upper bound on
    HW kernel time)."""
    import time

    import jax
    import jax.core
    from jax.sharding import Mesh, NamedSharding, PartitionSpec
    from jax.experimental.shard_map import shard_map

    import concourse.mybir as mybir
    from concourse import bass2jax

    nc, in_maps = _prepare(**np_inputs)
    bass2jax.install_neuronx_cc_hook()
    partition_name = nc.partition_id_tensor.name if nc.partition_id_tensor else None

    in_names, out_names, out_avals = [], [], []
    for alloc in nc.m.functions[0].allocations:
        if not isinstance(alloc, mybir.MemoryLocationSet):
            continue
        name = alloc.memorylocations[0].name
        if alloc.kind == "ExternalInput":
            if name != partition_name:
                in_names.append(name)
        elif alloc.kind == "ExternalOutput":
            out_names.append(name)
            out_avals.append(
                jax.core.ShapedArray(
                    tuple(alloc.tensor_shape), mybir.dt.np(alloc.dtype)
                )
            )
    all_names = in_names + out_names
    if partition_name is not None:
        all_names = all_names + [partition_name]

    def _body(*args):
        operands = list(args)
        if partition_name is not None:
            operands.append(bass2jax.partition_id_tensor())
        return tuple(
            bass2jax._bass_exec_p.bind(
                *operands,
                out_avals=tuple(out_avals),
                in_names=tuple(all_names),
                out_names=tuple(out_names),
                lowering_input_output_aliases=(),
                sim_require_finite=True,
                sim_require_nnan=True,
                nc=nc,
            )
        )

    devices = jax.devices()[:B]
    mesh = Mesh(np.array(devices), ("core",))
    nin = len(in_names) + len(out_names)
    sharded = jax.jit(
        shard_map(
            _body,
            mesh=mesh,
            in_specs=(PartitionSpec("core"),) * nin,
            out_specs=(PartitionSpec("core"),) * len(out_names),
            check_rep=False,
        ),
        keep_unused=True,
    )
    concat_in = [
        np.concatenate([np.asarray(in_maps[c][nm]) for c in range(B)], axis=0)
        for nm in in_names
    ]
    concat_zeros = [
        np.zeros((B * a.shape[0], *a.shape[1:]), a.dtype) for a in out_avals
    ]
    sharding = NamedSharding(mesh, PartitionSpec("core"))
    dev_args = [jax.device_put(a, sharding) for a in concat_in + concat_zeros]

    best = {}
    jax.block_until_ready(sharded(*dev_args))  # warm-up / compile
    for K in (k1, k2):
        b = None
        for _ in range(reps):
            t0 = time.perf_counter()
            outs = [sharded(*dev_args) for _ in range(K)]
            jax.block_until_ready(outs)
            dt = time.perf_counter() - t0
            b = dt if b is None else min(b, dt)
        best[K] = b
    return (best[k2] - best[k1]) / (k2 - k1) * 1e9


# revision 20
# speedup vs baseline: 1.8174x; 1.3455x over previous
"""Trainium2 Bass kernel for CausalBCNAttention.

Reference math (per batch b):
    Q = x @ W_Q^T ; K = x @ W_K^T
    A = Q @ V ; Kw = (K @ Wb) * mask
    S = cumsum_t(Kw) ; n = cumsum_t(mask)
    out = ((A*S) @ U^T + n*bias) / max(n,1)
    Y = out @ W_O^T

Algebraic refactoring (exact up to fp reassociation):
    A  = x @ (W_Q^T V)            -- WQV precomputed host-side
    Kw = (x*mask) @ (W_K^T Wb)    -- WKW precomputed host-side
    S  = cumsum_t(Kw)
    Y  = ((A/n) (.) S) @ (U^T W_O^T) + bias @ W_O^T
because the /n factor is a per-row scalar and commutes through right-matmuls.
So the device does 3 GEMMs of (4096x1024)@(1024x1024) per batch plus a
prefix-scan, instead of 6 GEMMs.

Device mapping: data-parallel over B across the 8 NeuronCores (core b owns
batch b). All tensors live "transposed" ([feature, t] with t on the free
dimension): the GEMM chain then needs no on-chip transposes and the causal
cumsum is a native DVE prefix scan (tensor_tensor_scan) along the free dim.
Matmuls run in bf16 with fp32 PSUM accumulation; the scan state is fp32.
"""

import numpy as np
import ml_dtypes

B, T, D, R = 8, 4096, 1024, 1024
P = 128
KO = D // P          # 8 contraction subtiles
RO = R // P          # 8 r tiles
EO = D // P          # 8 output-feature tiles
TB = 1024            # t megablock
NB = T // TB         # 4 megablocks
NC_CHUNK = 512       # matmul free-dim chunk (one PSUM bank)
CPB = TB // NC_CHUNK  # chunks per megablock

_bf16 = ml_dtypes.bfloat16

# Perf-experiment knobs (perf tooling overrides these module globals; the
# graded path always uses the defaults).
PSUM_BUFS = (8, 1, 1)
SCAN_AS_COPY = False
KW_FIRST = False
ST_BUFS = 2
INTERLEAVE_AKW = False
Y_ON_ACT = False
Z_ON_POOL = False
SHARED_PSUM = True
KW_C_OUTER = False


def _patch_tile_drain():
    """This container's walrus codegen rejects any instruction carrying more
    than one sync wait ("Too many sync wait commands"). Two fixes:
    (1) TileContext._add_instruction: hoist N>1 scheduler-attached waits onto
        N standalone NoOps emitted just before the instruction on the same
        engine stream (identical semantics: waits happen-before the op).
    (2) TileContext._drain_and_barrier: the tail drain aggregates waits for
        every active proc onto one Drain; split into one Drain per wait."""
    import bass_rust
    import concourse.mybir as mybir
    import concourse.tile as tile
    from concourse.vector_clock import ScopedClock

    if getattr(tile.TileContext, "_waitsplit_patched", False):
        return

    counter = [0]

    def _split_waits_add_instruction(self, inst):
        si = inst.sync_info
        if si is not None and len(si.on_wait) > 1:
            waits = list(si.on_wait)
            for w in waits:
                counter[0] += 1
                nop = mybir.InstNoOp(name=f"waitsplit-{counter[0]}")
                nop.engine = inst.engine
                nop.sync_info = bass_rust.SyncInfo(on_wait=[w], on_update=[])
                self.nc.register_instruction(nop, overwrite=True)
                self.nc.cur_bb.bb.add_instruction(nop)
            si.on_wait = []
            inst.sync_info = si
        self.nc.register_instruction(inst, overwrite=True)
        self.nc.cur_bb.bb.add_instruction(inst)

    def _split_drain_and_barrier(self, tick_clock, wait_clock):
        nc = self.nc
        drain_inst = nc.sync.drain()
        wait_clock.add_sem_waits(
            drain_inst.ins, ScopedClock({None: tick_clock.global_clock})
        )
        si = drain_inst.ins.sync_info
        waits = list(si.on_wait) if si is not None else []
        if len(waits) > 1:
            si.on_wait = waits[:1]
            drain_inst.ins.sync_info = si
            for w in waits[1:]:
                extra = nc.sync.drain()
                extra.ins.sync_info = bass_rust.SyncInfo(on_wait=[w], on_update=[])

        nc.all_engine_barrier()
        assert self.sems is not None
        popped = nc._tile_sem_poison_stack.pop()
        assert popped is self._sem_poison
        nc.clear_and_free_semaphores(list(self.sems.allocated().values()))
        nc.all_engine_barrier()

    tile.TileContext._add_instruction = _split_waits_add_instruction
    tile.TileContext._drain_and_barrier = _split_drain_and_barrier
    tile.TileContext._waitsplit_patched = True


def _build_program(dual_x: bool):
    """Trace the Bass/Tile program (identical on every core; SPMD)."""
    import concourse.bass as bass
    import concourse.mybir as mybir
    import concourse.tile as tile

    _patch_tile_drain()
    f32 = mybir.dt.float32
    bf16 = mybir.dt.bfloat16
    MUL = mybir.AluOpType.mult
    ADD = mybir.AluOpType.add
    BYP = mybir.AluOpType.bypass
    COPYF = mybir.ActivationFunctionType.Identity

    nc = bass.Bass()
    xt = nc.dram_tensor("xt", [D, T], bf16, kind="ExternalInput")
    xtk = (
        nc.dram_tensor("xtk", [D, T], bf16, kind="ExternalInput") if dual_x else xt
    )
    wa = nc.dram_tensor("wa", [D, R], bf16, kind="ExternalInput")
    wk = nc.dram_tensor("wk", [D, R], bf16, kind="ExternalInput")
    wy = nc.dram_tensor("wy", [R, D], bf16, kind="ExternalInput")
    invn = nc.dram_tensor("invn", [P, T], f32, kind="ExternalInput")
    ybias = nc.dram_tensor("ybias", [P, EO], f32, kind="ExternalInput")
    yt = nc.dram_tensor("yt", [D, T], f32, kind="ExternalOutput")

    xt_r = xt.rearrange("(ko ki) t -> ki ko t", ki=P)
    xtk_r = xtk.rearrange("(ko ki) t -> ki ko t", ki=P)
    wa_r = wa.rearrange("(ko ki) r -> ki ko r", ki=P)
    wk_r = wk.rearrange("(ko ki) r -> ki ko r", ki=P)
    wy_r = wy.rearrange("(ro ri) e -> ri ro e", ri=P)
    yt_r = yt.rearrange("(eo ei) t -> ei eo t", ei=P)

    with tile.TileContext(nc) as tc:
        with (
            tc.tile_pool(name="weights", bufs=1) as wpool,
            tc.tile_pool(name="consts", bufs=1) as cpool,
            tc.tile_pool(name="xin", bufs=2) as xpool,
            tc.tile_pool(name="xkin", bufs=2 if dual_x else 1) as xkpool,
            # dual_x adds 32KB/partition of xtk tiles; shrink the A' pool to
            # stay under the 192KB/partition SBUF budget on that (cold) path.
            tc.tile_pool(name="aprime", bufs=1 if dual_x else 2) as apool,
            tc.tile_pool(name="zbuf", bufs=2) as zpool,
            tc.tile_pool(name="stbuf", bufs=ST_BUFS) as stpool,
            tc.tile_pool(name="carry", bufs=1) as crpool,
            tc.tile_pool(name="yout", bufs=2 if dual_x else 4) as ypool,
            tc.tile_pool(name="pa", bufs=PSUM_BUFS[0], space="PSUM") as pa_pool,
            tc.tile_pool(name="pk", bufs=PSUM_BUFS[1], space="PSUM") as pk_pool,
            tc.tile_pool(name="py", bufs=PSUM_BUFS[2], space="PSUM") as py_pool,
        ):
            if SHARED_PSUM:
                pk_pool = pa_pool
                py_pool = pa_pool
            # Preamble loads are sliced per contraction subtile so the first
            # matmuls (which need only wa[:, 0] + xt0[:, 0]) can start after
            # <1 MB of DMA instead of after every weight has landed.
            wa_sb = wpool.tile([P, KO, R], bf16, tag="wa")
            wk_sb = wpool.tile([P, KO, R], bf16, tag="wk")
            wy_sb = wpool.tile([P, RO, D], bf16, tag="wy")
            xt0 = xpool.tile([P, KO, TB], bf16, tag="xt", name="xt0")
            invn_sb = cpool.tile([P, T], f32, tag="invn")
            for d in range(KO):
                nc.sync.dma_start(wa_sb[:, d], wa_r[:, d])
                nc.sync.dma_start(xt0[:, d], xt_r[:, d, 0:TB])
                nc.sync.dma_start(wk_sb[:, d], wk_r[:, d])
                nc.sync.dma_start(
                    invn_sb[:, d * (T // KO) : (d + 1) * (T // KO)],
                    invn[:, d * (T // KO) : (d + 1) * (T // KO)],
                )
            nc.sync.dma_start(wy_sb[:], wy_r[:])
            ybias_sb = cpool.tile([P, EO], f32, tag="ybias")
            nc.sync.dma_start(ybias_sb[:], ybias[:])
            carry_sb = crpool.tile([P, RO], f32, tag="carry")

            for mb in range(NB):
                t0 = mb * TB

                if mb == 0:
                    xt_mb = xt0
                else:
                    xt_mb = xpool.tile([P, KO, TB], bf16, tag="xt")
                    nc.sync.dma_start(xt_mb[:], xt_r[:, :, t0 : t0 + TB])
                if dual_x:
                    xtk_mb = xkpool.tile([P, KO, TB], bf16, tag="xtk")
                    nc.sync.dma_start(xtk_mb[:], xtk_r[:, :, t0 : t0 + TB])
                else:
                    xtk_mb = xt_mb

                # ---- A' stage: A'[r,t] = (x @ WQV)^T[r,t] * invn[t] ----
                at_mb = apool.tile([P, RO, TB], bf16, tag="at")

                def a_stage(ro):
                    ps = [
                        pa_pool.tile([P, NC_CHUNK], f32, tag="ps" if SHARED_PSUM else "pa", name=f"pa{c}") for c in range(CPB)
                    ]
                    for d in range(KO):
                        for c in range(CPB):
                            nc.tensor.matmul(
                                ps[c][:],
                                wa_sb[:, d, ro * P : (ro + 1) * P],
                                xt_mb[:, d, c * NC_CHUNK : (c + 1) * NC_CHUNK],
                                start=(d == 0),
                                stop=(d == KO - 1),
                            )
                    for c in range(CPB):
                        cs = slice(c * NC_CHUNK, (c + 1) * NC_CHUNK)
                        gs = slice(t0 + c * NC_CHUNK, t0 + (c + 1) * NC_CHUNK)
                        nc.vector.tensor_tensor(
                            at_mb[:, ro, cs], ps[c][:], invn_sb[:, gs], MUL
                        )

                # ---- Kw stage + causal prefix scan + Z ----
                zt_mb = zpool.tile([P, RO, TB], bf16, tag="zt")

                def kw_stage(ro):
                    ps = [
                        pk_pool.tile([P, NC_CHUNK], f32, tag="ps" if SHARED_PSUM else "pk", name=f"pk{c}") for c in range(CPB)
                    ]
                    if KW_C_OUTER:
                        for c in range(CPB):
                            for d in range(KO):
                                nc.tensor.matmul(
                                    ps[c][:],
                                    wk_sb[:, d, ro * P : (ro + 1) * P],
                                    xtk_mb[:, d, c * NC_CHUNK : (c + 1) * NC_CHUNK],
                                    start=(d == 0),
                                    stop=(d == KO - 1),
                                )
                    else:
                        for d in range(KO):
                            for c in range(CPB):
                                nc.tensor.matmul(
                                    ps[c][:],
                                    wk_sb[:, d, ro * P : (ro + 1) * P],
                                    xtk_mb[:, d, c * NC_CHUNK : (c + 1) * NC_CHUNK],
                                    start=(d == 0),
                                    stop=(d == KO - 1),
                                )
                    st = stpool.tile([P, TB], f32, tag="st")
                    for c in range(CPB):
                        cs = slice(c * NC_CHUNK, (c + 1) * NC_CHUNK)
                        gs = slice(t0 + c * NC_CHUNK, t0 + (c + 1) * NC_CHUNK)
                        if SCAN_AS_COPY:
                            # timing-control variant: same data movement, no
                            # recurrence (WRONG math; perf probing only)
                            nc.vector.tensor_copy(st[:, cs], ps[c][:])
                        else:
                            nc.vector.tensor_tensor_scan(
                                st[:, cs],
                                ps[c][:],
                                invn_sb[:, gs],
                                (
                                    (0.0 if mb == 0 else carry_sb[:, ro : ro + 1])
                                    if c == 0
                                    else st[:, c * NC_CHUNK - 1 : c * NC_CHUNK]
                                ),
                                ADD,
                                BYP,
                            )
                    nc.vector.tensor_copy(
                        carry_sb[:, ro : ro + 1], st[:, TB - 1 : TB]
                    )
                    if Z_ON_POOL:
                        nc.gpsimd.tensor_tensor(
                            zt_mb[:, ro, :], st[:], at_mb[:, ro, :], MUL
                        )
                    else:
                        nc.vector.tensor_tensor(
                            zt_mb[:, ro, :], st[:], at_mb[:, ro, :], MUL
                        )

                if INTERLEAVE_AKW:
                    for ro in range(RO):
                        a_stage(ro)
                        kw_stage(ro)
                elif KW_FIRST:
                    for ro in range(RO):
                        kw_stage(ro)
                    for ro in range(RO):
                        a_stage(ro)
                else:
                    for ro in range(RO):
                        a_stage(ro)
                    for ro in range(RO):
                        kw_stage(ro)

                # ---- Y stage: Y^T[e,t] = (Z^T)^T-contraction with UW + ybias ----
                for eo in range(EO):
                    ps = [
                        py_pool.tile([P, NC_CHUNK], f32, tag="ps" if SHARED_PSUM else "py", name=f"py{c}") for c in range(CPB)
                    ]
                    for r in range(RO):
                        for c in range(CPB):
                            nc.tensor.matmul(
                                ps[c][:],
                                wy_sb[:, r, eo * P : (eo + 1) * P],
                                zt_mb[:, r, c * NC_CHUNK : (c + 1) * NC_CHUNK],
                                start=(r == 0),
                                stop=(r == RO - 1),
                            )
                    for c in range(CPB):
                        ytile = ypool.tile([P, NC_CHUNK], f32, tag="yt")
                        if Y_ON_ACT:
                            nc.scalar.activation(
                                ytile[:], ps[c][:], COPYF,
                                bias=ybias_sb[:, eo : eo + 1],
                            )
                        else:
                            nc.vector.tensor_scalar_add(
                                ytile[:], ps[c][:], ybias_sb[:, eo : eo + 1]
                            )
                        nc.sync.dma_start(
                            yt_r[
                                :,
                                eo,
                                t0 + c * NC_CHUNK : t0 + (c + 1) * NC_CHUNK,
                            ],
                            ytile[:],
                        )
    return nc


def _prepare(x, attention_mask, W_Q, W_K, U, V, Wb, bias, W_O):
    x = np.asarray(x, dtype=np.float32)
    attention_mask = np.asarray(attention_mask)
    W_Q = np.asarray(W_Q, dtype=np.float32)
    W_K = np.asarray(W_K, dtype=np.float32)
    U = np.asarray(U, dtype=np.float32)
    V = np.asarray(V, dtype=np.float32)
    Wb = np.asarray(Wb, dtype=np.float32)
    bias = np.asarray(bias, dtype=np.float32)
    W_O = np.asarray(W_O, dtype=np.float32)

    m = attention_mask.astype(np.float32)          # (B,T)
    all_ones = bool(np.all(m == 1.0))

    # Host-side weight folding (exact math, fp32).
    WQV = (W_Q.T @ V).astype(_bf16)                # (D,R)
    WKW = (W_K.T @ Wb).astype(_bf16)               # (D,R)
    UW = (W_O @ U).T.astype(_bf16)                 # (R,D)
    ybias_e = (W_O @ bias).astype(np.float32)      # (D,)
    ybias_tile = np.ascontiguousarray(
        ybias_e.reshape(EO, P).T
    )                                              # (P, EO) [ei, eo]

    n = np.cumsum(m, axis=1)                       # (B,T)
    invn = (1.0 / np.clip(n, 1.0, None)).astype(np.float32)

    nc = _build_program(dual_x=not all_ones)

    in_maps = []
    for b in range(B):
        xt_b = np.ascontiguousarray(x[b].T).astype(_bf16)     # (D,T)
        im = {
            "xt": xt_b,
            "wa": WQV,
            "wk": WKW,
            "wy": UW,
            "invn": np.ascontiguousarray(
                np.broadcast_to(invn[b][None, :], (P, T))
            ),
            "ybias": ybias_tile,
        }
        if not all_ones:
            im["xtk"] = np.ascontiguousarray(
                (x[b] * m[b][:, None]).T
            ).astype(_bf16)
        in_maps.append(im)

    return nc, in_maps


def _gather(res):
    out = np.empty((B, T, D), dtype=np.float32)
    for b in range(B):
        out[b] = res.results[b]["yt"].T
    return out


def kernel(x, attention_mask, W_Q, W_K, U, V, Wb, bias, W_O):
    from concourse.bass_utils import run_bass_kernel_spmd

    nc, in_maps = _prepare(x, attention_mask, W_Q, W_K, U, V, Wb, bias, W_O)
    res = run_bass_kernel_spmd(nc, in_maps, core_ids=list(range(B)))
    return _gather(res)


def run_timed(np_inputs, k1=16, k2=64, reps=5):
    """Estimate per-execution device time.

    This axon build has no NTFF profiling hook, so we measure the marginal
    wall-clock of asynchronously queued executions: dispatch K in flight,
    block once; slope between K=k1 and K=k2 amortizes the ~0.5 ms
    per-dispatch relay overhead. Returns ns per execution (upper bound on
    HW kernel time)."""
    import time

    import jax
    import jax.core
    from jax.sharding import Mesh, NamedSharding, PartitionSpec
    from jax.experimental.shard_map import shard_map

    import concourse.mybir as mybir
    from concourse import bass2jax

    nc, in_maps = _prepare(**np_inputs)
    bass2jax.install_neuronx_cc_hook()
    partition_name = nc.partition_id_tensor.name if nc.partition_id_tensor else None

    in_names, out_names, out_avals = [], [], []
    for alloc in nc.m.functions[0].allocations:
        if not isinstance(alloc, mybir.MemoryLocationSet):
            continue
        name = alloc.memorylocations[0].name
        if alloc.kind == "ExternalInput":
            if name != partition_name:
                in_names.append(name)
        elif alloc.kind == "ExternalOutput":
            out_names.append(name)
            out_avals.append(
                jax.core.ShapedArray(
                    tuple(alloc.tensor_shape), mybir.dt.np(alloc.dtype)
                )
            )
    all_names = in_names + out_names
    if partition_name is not None:
        all_names = all_names + [partition_name]

    def _body(*args):
        operands = list(args)
        if partition_name is not None:
            operands.append(bass2jax.partition_id_tensor())
        return tuple(
            bass2jax._bass_exec_p.bind(
                *operands,
                out_avals=tuple(out_avals),
                in_names=tuple(all_names),
                out_names=tuple(out_names),
                lowering_input_output_aliases=(),
                sim_require_finite=True,
                sim_require_nnan=True,
                nc=nc,
            )
        )

    devices = jax.devices()[:B]
    mesh = Mesh(np.array(devices), ("core",))
    nin = len(in_names) + len(out_names)
    sharded = jax.jit(
        shard_map(
            _body,
            mesh=mesh,
            in_specs=(PartitionSpec("core"),) * nin,
            out_specs=(PartitionSpec("core"),) * len(out_names),
            check_rep=False,
        ),
        keep_unused=True,
    )
    concat_in = [
        np.concatenate([np.asarray(in_maps[c][nm]) for c in range(B)], axis=0)
        for nm in in_names
    ]
    concat_zeros = [
        np.zeros((B * a.shape[0], *a.shape[1:]), a.dtype) for a in out_avals
    ]
    sharding = NamedSharding(mesh, PartitionSpec("core"))
    dev_args = [jax.device_put(a, sharding) for a in concat_in + concat_zeros]

    best = {}
    jax.block_until_ready(sharded(*dev_args))  # warm-up / compile
    for K in (k1, k2):
        b = None
        for _ in range(reps):
            t0 = time.perf_counter()
            outs = [sharded(*dev_args) for _ in range(K)]
            jax.block_until_ready(outs)
            dt = time.perf_counter() - t0
            b = dt if b is None else min(b, dt)
        best[K] = b
    return (best[k2] - best[k1]) / (k2 - k1) * 1e9
